# revision 19
# baseline (speedup 1.0000x reference)
"""MultiHeadLatentAttention Trainium2 Bass kernel (v2 - fused bf16 pipeline).

Sharding (8 cores): core c = (b, hg) with b = c // 2, hg = c % 2.
Each core handles batch b and head-group hg (8 of 16 heads):
  - fused per-512-token-chunk pipeline: QKV proj (bf16 matmuls) ->
    rmsnorm (DVE reduce + ACT ln/exp rsqrt) -> RoPE (DVE) -> XBAR
    DMA-transpose into SBUF-resident q^T/k^T tiles -> causal attention
    (batched ACT exp, bf16 matmuls) -> per-chunk pairwise y^T exchange
    (AllGather) -> out-projection for c-half hg*1024:(hg+1)*1024.
Host concatenates the two c-halves per batch. Host-side work is only
slicing/transposing/casting weights and building constant tables.
"""

import numpy as np
import ml_dtypes

import concourse.bass as bass
import concourse.mybir as mybir
import concourse.tile as tile
from concourse import bacc
from concourse.bass import ts

F32 = mybir.dt.float32
F32R = mybir.dt.float32r
BF16 = mybir.dt.bfloat16
AF = mybir.ActivationFunctionType

N_HEAD = 16
N_EMBD = 2048
N_LATENT = 1024
HEAD_DIM = 64
ROPE_BASE = 10000.0
EPS = 1e-6
N_CORES = 8

HPC = N_HEAD // 2        # heads per core = 8
DW = HPC * HEAD_DIM      # local head width = 512
TCH = 512                # token chunk for the fused pipeline
SCALE = 1.0 / np.sqrt(HEAD_DIM)


def build_nc(T=2048, C=2048, num_devices=N_CORES, exchange="cc", reps=1,
             debug_out=False):
    """Build the SPMD program (identical on all cores; data differs).

    exchange: "cc" = pairwise AllGather collectives, "none" = skip exchange
    (timing-only; out-proj reads the local half twice).
    """
    nc = bacc.Bacc("TRN2", target_bir_lowering=False, debug=False,
                   num_devices=num_devices)

    NT = T // 128            # t-tiles = 16
    NCT = C // 128           # contraction tiles for qkv proj = 16
    NJ = T // TCH            # chunks = 4
    TPC = TCH // 128         # t-tiles per chunk = 4
    CH = C // 2              # out c-half width = 1024
    NL = N_LATENT // 128     # latent 128-blocks = 8
    CCW = 512                # out column chunk
    NCC = CH // CCW          # = 2

    x_d = nc.dram_tensor("x", [T, C], F32, kind="ExternalInput").ap()
    wq_d = nc.dram_tensor("wqT", [C, DW], BF16, kind="ExternalInput").ap()
    wk_d = nc.dram_tensor("wkT", [C, DW], BF16, kind="ExternalInput").ap()
    wv_d = nc.dram_tensor("wvT", [C, DW], BF16, kind="ExternalInput").ap()
    wo_d = nc.dram_tensor("woutT", [N_LATENT, CH], BF16,
                          kind="ExternalInput").ap()
    cos_d = nc.dram_tensor("cosf", [T, DW], BF16, kind="ExternalInput").ap()
    sin_d = nc.dram_tensor("sinf", [T, DW], BF16, kind="ExternalInput").ap()
    mask_d = nc.dram_tensor("masks", [4, 128, TCH], BF16,
                            kind="ExternalInput").ap()
    out_d = nc.dram_tensor("out_half", [T, CH], F32, kind="ExternalOutput").ap()
    dbg = {}
    if debug_out:
        for nm, shp in (("qt_o", [DW, T]), ("kt_o", [DW, T]),
                        ("v_o", [T, DW]), ("yt_o", [DW, T])):
            dbg[nm] = nc.dram_tensor(nm, shp, BF16, kind="ExternalOutput").ap()

    groups = [[i, i + 1] for i in range(0, num_devices, 2)]
    exchange_mode = exchange

    with tile.TileContext(nc) as tc:
        with (
            tc.tile_pool(name="const", bufs=1) as constp,
            tc.tile_pool(name="dram", bufs=1, space=bass.MemorySpace.DRAM) as dramp,
            tc.tile_pool(name="wp", bufs=1) as wp,
            tc.tile_pool(name="kvp", bufs=1) as kvp,        # ktt + v65 (persist)
            tc.tile_pool(name="qp", bufs=2) as qp,          # q^T per chunk
            tc.tile_pool(name="p1x", bufs=2) as p1x,        # x staging
            tc.tile_pool(name="p1s", bufs=2) as p1s,        # rope staging
            tc.tile_pool(name="p2s", bufs=2) as p2s,        # pt + att staging
            tc.tile_pool(name="p3y", bufs=1) as p3y,
            tc.tile_pool(name="p3o", bufs=3) as p3o,        # yfull + osb
            tc.tile_pool(name="ps512", bufs=2,
                         space=bass.MemorySpace.PSUM) as ps512,
            tc.tile_pool(name="pssp", bufs=2,
                         space=bass.MemorySpace.PSUM) as pssp,
            tc.tile_pool(name="pysp", bufs=1,
                         space=bass.MemorySpace.PSUM) as pysp,
        ):
            # ---------------- constants + weights ----------------
            eps_sb = constp.tile([128, 1], F32, tag="eps")
            nc.vector.memset(eps_sb[:], EPS)
            mask_sb = constp.tile([128, 4, TCH], BF16, tag="masks")
            nc.sync.dma_start(mask_sb[:], mask_d.rearrange("o p t -> p o t"))

            wsb = {}
            for name, wd in (("q", wq_d), ("k", wk_d), ("v", wv_d)):
                w = wp.tile([128, NCT, DW], BF16, tag=f"w{name}",
                            name=f"w{name}")
                nc.sync.dma_start(
                    w[:], wd.rearrange("(ct p) d -> p ct d", p=128))
                wsb[name] = w
            wo = wp.tile([128, NL, CH], BF16, tag="wo")
            nc.sync.dma_start(wo[:], wo_d.rearrange("(lt p) c -> p lt c",
                                                    p=128))

            # persistent k^T / v65 tiles (one per 128-token block)
            ktts = [kvp.tile([128, HPC // 2, 128], BF16, tag=f"ktt{si}",
                             name=f"ktt{si}") for si in range(NT)]
            v65s = []
            for si in range(NT):
                v = kvp.tile([128, HPC, 65], BF16, tag=f"v65_{si}",
                             name=f"v65_{si}")
                nc.vector.memset(
                    v[:, :, 64:65].rearrange("p h one -> p (h one)"), 1.0)
                v65s.append(v)

            # y^T exchange buffers (DRAM, bf16)
            ytls = [dramp.tile([DW, TCH], BF16, tag=f"ytl{j}",
                               name=f"ytl{j}") for j in range(NJ)]
            ytfs = [dramp.tile([2 * DW, TCH], BF16, tag=f"ytf{j}",
                               name=f"ytf{j}") for j in range(NJ)]
            ytfhs = [dramp.tile([256, TCH], BF16, tag=f"ytfh{hp}",
                                name=f"ytfh{hp}") for hp in range(HPC // 2)]

            qtcs = [None, None]

            # ---------------- phase bodies ----------------
            def p1_tile(tt):
                """QKV + rmsnorm + rope for t-tile tt; fills ktt/v65/qtc."""
                xa = p1x.tile([128, C], F32, tag="xa")
                nc.sync.dma_start(xa[:], x_d[ts(tt, 128), :])
                xab = p1x.tile([128, C], BF16, tag="xab")
                nc.scalar.activation(xab[:], xa[:], AF.Copy)
                xt = p1x.tile([128, NCT, 128], BF16, tag="xt")
                nc.sync.dma_start_transpose(xt[:], xab[:])
                cos_t = p1x.tile([128, DW], BF16, tag="cos")
                sin_t = p1x.tile([128, DW], BF16, tag="sin")
                nc.sync.dma_start(cos_t[:], cos_d[ts(tt, 128), :])
                nc.sync.dma_start(sin_t[:], sin_d[ts(tt, 128), :])

                # q and k processed jointly as [128, 2, DW] (g = q|k)
                qkb = p1s.tile([128, 2, DW], BF16, tag="qkb")
                for gi, name in enumerate(("q", "k", "v")):
                    ps = ps512.tile([128, DW], F32, tag="ps512")
                    for ct in range(NCT):
                        nc.tensor.matmul(
                            ps[:], xt[:, ct, :], wsb[name][:, ct, :],
                            start=(ct == 0), stop=(ct == NCT - 1))
                    if name == "v":
                        nc.scalar.activation(
                            v65s[tt][:, :, 0:64], ps[:], AF.Copy)
                    else:
                        nc.scalar.activation(qkb[:, gi, :], ps[:], AF.Copy)
                # rmsnorm factors: rf = exp(-0.5*ln(ms/64 + eps))
                sq = p1s.tile([128, 2, DW], BF16, tag="sq")
                nc.vector.tensor_mul(sq[:], qkb[:], qkb[:])
                ms = p1s.tile([128, 2 * HPC], F32, tag="ms")
                nc.vector.tensor_reduce(
                    ms[:], sq[:].rearrange("p g (h d) -> p (g h) d",
                                           d=HEAD_DIM),
                    axis=mybir.AxisListType.X, op=mybir.AluOpType.add)
                # rf = rsqrt(ms/64 + eps): Quake seed + 3 Newton steps,
                # all single-op DVE tensor_scalar (validated sequence)
                nc.vector.tensor_scalar(ms[:], ms[:], 1.0 / HEAD_DIM, None,
                                        mybir.AluOpType.mult)
                nc.vector.tensor_scalar(ms[:], ms[:], EPS, None,
                                        mybir.AluOpType.add)
                rfi = p1s.tile([128, 2 * HPC], mybir.dt.int32, tag="rfi")
                nc.vector.tensor_scalar(rfi[:], ms[:].bitcast(mybir.dt.int32),
                                        1, None,
                                        mybir.AluOpType.logical_shift_right)
                nc.vector.tensor_scalar(rfi[:], rfi[:], -1, None,
                                        mybir.AluOpType.mult)
                nc.vector.tensor_scalar(rfi[:], rfi[:], 0x5F3759DF, None,
                                        mybir.AluOpType.add)
                rf = p1s.tile([128, 2 * HPC], F32, tag="rf")
                rfv = rfi[:].bitcast(F32)
                nt = p1s.tile([128, 2 * HPC], F32, tag="nt")
                for _ in range(3):
                    nc.vector.tensor_mul(nt[:], rfv, rfv)
                    nc.vector.tensor_mul(nt[:], nt[:], ms[:])
                    nc.vector.tensor_scalar(nt[:], nt[:], -0.5, None,
                                            mybir.AluOpType.mult)
                    nc.vector.tensor_scalar(nt[:], nt[:], 1.5, None,
                                            mybir.AluOpType.add)
                    nc.vector.tensor_mul(rfv, rfv, nt[:])
                nc.vector.tensor_copy(rf[:], rfv)
                qkn = p1s.tile([128, 2, DW], BF16, tag="qkn")
                rfb = (rf[:].rearrange("p (g h one) -> p g h one", g=2, one=1)
                       .broadcast_to([128, 2, HPC, HEAD_DIM]))
                nc.vector.tensor_mul(
                    qkn[:].rearrange("p g (h d) -> p g h d", d=HEAD_DIM),
                    qkb[:].rearrange("p g (h d) -> p g h d", d=HEAD_DIM), rfb)
                # rope
                qks = p1s.tile([128, 2, DW], BF16, tag="qks")
                hv = qkn[:].rearrange("p g (h two d) -> p g h two d", two=2,
                                      d=HEAD_DIM // 2)
                sv = qks[:].rearrange("p g (h two d) -> p g h two d", two=2,
                                      d=HEAD_DIM // 2)
                nc.vector.tensor_copy(sv[:, :, :, 0, :], hv[:, :, :, 1, :])
                nc.vector.tensor_copy(sv[:, :, :, 1, :], hv[:, :, :, 0, :])
                m1 = p1s.tile([128, 2, DW], BF16, tag="m1")
                m2 = p1s.tile([128, 2, DW], BF16, tag="m2")
                cosb = (cos_t[:].rearrange("p (one d) -> p one d", one=1)
                        .broadcast_to([128, 2, DW]))
                sinb = (sin_t[:].rearrange("p (one d) -> p one d", one=1)
                        .broadcast_to([128, 2, DW]))
                nc.vector.tensor_mul(m1[:], qkn[:], cosb)
                nc.vector.tensor_mul(m2[:], qks[:], sinb)
                nc.vector.tensor_add(m1[:], m1[:], m2[:])
                j, ttl = tt // TPC, tt % TPC
                nc.sync.dma_start_transpose(
                    qtcs[j % 2][:, :, ts(ttl, 128)], m1[:, 0, :])
                nc.sync.dma_start_transpose(ktts[tt][:], m1[:, 1, :])

            def attention(j, hp):
                smax = TPC * (j + 1)
                pys = [pysp.tile([65, TCH], F32, tag=f"py{e}", name=f"py{e}")
                       for e in range(2)]
                for si in range(smax):
                    pss = pssp.tile([128, 2, TCH], F32, tag="pss")
                    for e in range(2):
                        nc.tensor.matmul(
                            pss[:, e, :],
                            ktts[si][ts(e, 64), hp, :],
                            qtcs[j % 2][ts(e, 64), hp, :],
                            start=True, stop=True)
                    pt = p2s.tile([128, 2, TCH], BF16, tag="pt")
                    nc.scalar.activation(
                        pt[:].rearrange("p a b -> p (a b)"),
                        pss[:].rearrange("p a b -> p (a b)"),
                        AF.Exp, scale=SCALE)
                    o = si - (smax - TPC)
                    if o >= 0:
                        for e in range(2):
                            nc.vector.tensor_mul(pt[:, e, :], pt[:, e, :],
                                                 mask_sb[:, o, :])
                    for e in range(2):
                        nc.tensor.matmul(
                            pys[e][:], v65s[si][:, 2 * hp + e, :], pt[:, e, :],
                            start=(si == 0), stop=(si == smax - 1))
                for e in range(2):
                    ystage = p2s.tile([65, TCH], F32, tag="ystage", bufs=2)
                    nc.vector.tensor_copy(ystage[:], pys[e][:])
                    bcr1 = p2s.tile([1, TCH], F32, tag="bcr1", bufs=1)
                    nc.vector.reciprocal(bcr1[:], ystage[64:65, :])
                    bc64 = p2s.tile([64, TCH], F32, tag="bc64", bufs=1)
                    nc.gpsimd.partition_broadcast(bc64[:], bcr1[:])
                    ynt = p2s.tile([64, TCH], BF16, tag="ynt", bufs=2)
                    nc.vector.tensor_mul(ynt[:], ystage[0:64, :], bc64[:])
                    nc.sync.dma_start(
                        ytls[j][ts(2 * hp + e, HEAD_DIM), :], ynt[:])

            def do_exchange(j):
                if exchange_mode == "cc":
                    nc.gpsimd.collective_compute(
                        "AllGather", mybir.AluOpType.bypass,
                        replica_groups=groups,
                        ins=[ytls[j][:]],
                        outs=[ytfs[j][:]])

            def do_exchange_hp(j, hp):
                if exchange_mode == "cc":
                    nc.gpsimd.collective_compute(
                        "AllGather", mybir.AluOpType.bypass,
                        replica_groups=groups,
                        ins=[ytls[j][ts(hp, 128), :]],
                        outs=[ytfhs[hp][:]])

            def p3_chunk(j):
                yf = p3y.tile([128, NL, TCH], BF16, tag="yf")
                if exchange_mode == "cc" and j == NJ - 1:
                    for hp in range(HPC // 2):
                        for h2 in range(2):
                            nc.sync.dma_start(
                                yf[:, h2 * (HPC // 2) + hp, :],
                                ytfhs[hp][ts(h2, 128), :])
                elif exchange_mode == "cc":
                    nc.sync.dma_start(
                        yf[:], ytfs[j][:].rearrange("(lt p) t -> p lt t",
                                                    p=128))
                else:
                    for half in range(2):
                        nc.sync.dma_start(
                            yf[:, half * (NL // 2):(half + 1) * (NL // 2), :],
                            ytls[j][:].rearrange(
                                "(lt p) t -> p lt t", p=128))
                for ttl in range(TPC):
                    tt = j * TPC + ttl
                    for cc in range(NCC):
                        po = ps512.tile([128, CCW], F32, tag="ps512")
                        for lt in range(NL):
                            nc.tensor.matmul(
                                po[:], yf[:, lt, ts(ttl, 128)],
                                wo[:, lt, ts(cc, CCW)],
                                start=(lt == 0), stop=(lt == NL - 1))
                        osb = p3o.tile([128, CCW], F32, tag="osb")
                        nc.vector.tensor_copy(osb[:], po[:])
                        nc.sync.dma_start(out_d[ts(tt, 128), ts(cc, CCW)],
                                          osb[:])

            for _rep in range(reps):
                qtcs[0] = qp.tile([128, HPC // 2, TCH], BF16, tag="qtc0",
                                  name=f"qtc0_{_rep}")
                qtcs[1] = qp.tile([128, HPC // 2, TCH], BF16, tag="qtc1",
                                  name=f"qtc1_{_rep}")
                # Order keeps all DMA-transposes of chunk j+1 ahead of
                # exchange(j): tile serializes collectives against XBAR
                # transposes, so transposes emitted after a collective would
                # stall the P1 pipeline behind it.
                for ttl in range(TPC):
                    p1_tile(ttl)
                for j in range(NJ):
                    for hp in range(HPC // 2):
                        attention(j, hp)
                        if j == NJ - 1:
                            do_exchange_hp(j, hp)
                    if j + 1 < NJ:
                        for ttl in range(TPC):
                            p1_tile((j + 1) * TPC + ttl)
                    if j >= 1:
                        p3_chunk(j - 1)
                    if j < NJ - 1:
                        do_exchange(j)
                p3_chunk(NJ - 1)

                if debug_out:
                    for si in range(NT):
                        nc.gpsimd.dma_start(
                            dbg["kt_o"].rearrange(
                                "(hp p) t -> p hp t", p=128)[:, :, ts(si, 128)],
                            ktts[si][:])
                        nc.gpsimd.dma_start(
                            dbg["v_o"][ts(si, 128), :].rearrange(
                                "p (h d) -> p h d", d=HEAD_DIM),
                            v65s[si][:, :, 0:64])
                    for m in range(2):
                        nc.gpsimd.dma_start(
                            dbg["qt_o"].rearrange(
                                "(hp p) t -> p hp t",
                                p=128)[:, :, ts(2 + m, TCH)],
                            qtcs[m][:])
                    for j in range(NJ):
                        nc.gpsimd.dma_start(dbg["yt_o"][:, ts(j, TCH)],
                                            ytls[j][:])

    nc.compile()
    return nc


def host_tables(T=2048):
    inv_freq = 1.0 / (ROPE_BASE ** (np.arange(0, HEAD_DIM, 2, dtype=np.float32)
                                    / HEAD_DIM))
    t = np.arange(T, dtype=np.float32)
    freqs = np.outer(t, inv_freq)
    cos = np.cos(freqs).astype(np.float32)
    sin = np.sin(freqs).astype(np.float32)
    cosf = np.tile(np.concatenate([cos, cos], axis=1), (1, HPC))
    sinf = np.tile(np.concatenate([sin, -sin], axis=1), (1, HPC))
    masks = np.zeros((4, 128, TCH), dtype=np.float32)
    for i, o in enumerate(range(0, TCH, 128)):
        masks[i] = (np.arange(TCH)[None, :] >=
                    (np.arange(128)[:, None] + o)).astype(np.float32)
    bf = ml_dtypes.bfloat16
    return (np.ascontiguousarray(cosf).astype(bf),
            np.ascontiguousarray(sinf).astype(bf),
            masks.astype(bf))


def make_in_maps(x, w_qkv, w_out, T=2048, num_devices=N_CORES):
    bf = ml_dtypes.bfloat16
    x = np.asarray(x, dtype=np.float32)
    w_qkv = np.asarray(w_qkv, dtype=np.float32)
    w_out = np.asarray(w_out, dtype=np.float32)
    C = x.shape[-1]
    cosf, sinf, masks = host_tables(T)
    in_maps = []
    for c in range(num_devices):
        b, hg = c // 2, c % 2
        sl = slice(hg * DW, (hg + 1) * DW)
        in_maps.append({
            "x": np.ascontiguousarray(x[b]),
            "wqT": np.ascontiguousarray(w_qkv[0 * N_LATENT:, :][sl].T).astype(bf),
            "wkT": np.ascontiguousarray(w_qkv[1 * N_LATENT:, :][sl].T).astype(bf),
            "wvT": np.ascontiguousarray(w_qkv[2 * N_LATENT:, :][sl].T).astype(bf),
            "woutT": np.ascontiguousarray(
                w_out[hg * C // 2:(hg + 1) * C // 2, :].T).astype(bf),
            "cosf": cosf,
            "sinf": sinf,
            "masks": masks,
        })
    return in_maps


_NC = None


def kernel(x, w_qkv, w_out):
    global _NC
    if _NC is None:
        _NC = build_nc()
    from concourse.bass_utils import run_bass_kernel_spmd
    in_maps = make_in_maps(x, w_qkv, w_out)
    res = run_bass_kernel_spmd(_NC, in_maps, list(range(N_CORES))).results
    B, T = 4, 2048
    out = np.empty((B, T, N_EMBD), dtype=np.float32)
    for c in range(N_CORES):
        b, hg = c // 2, c % 2
        out[b, :, hg * N_EMBD // 2:(hg + 1) * N_EMBD // 2] = res[c]["out_half"]
    return out


# revision 20
# speedup vs baseline: 1022934.0000x; 1022934.0000x over previous
"""MultiHeadLatentAttention Trainium2 Bass kernel (v2 - fused bf16 pipeline).

Sharding (8 cores): core c = (b, hg) with b = c // 2, hg = c % 2.
Each core handles batch b and head-group hg (8 of 16 heads):
  - fused per-512-token-chunk pipeline: QKV proj (bf16 matmuls) ->
    rmsnorm (DVE reduce + ACT ln/exp rsqrt) -> RoPE (DVE) -> XBAR
    DMA-transpose into SBUF-resident q^T/k^T tiles -> causal attention
    (batched ACT exp, bf16 matmuls) -> per-chunk pairwise y^T exchange
    (AllGather) -> out-projection for c-half hg*1024:(hg+1)*1024.
Host concatenates the two c-halves per batch. Host-side work is only
slicing/transposing/casting weights and building constant tables.
"""

import numpy as np
import ml_dtypes

import concourse.bass as bass
import concourse.mybir as mybir
import concourse.tile as tile
from concourse import bacc
from concourse.bass import ts

F32 = mybir.dt.float32
F32R = mybir.dt.float32r
BF16 = mybir.dt.bfloat16
AF = mybir.ActivationFunctionType

N_HEAD = 16
N_EMBD = 2048
N_LATENT = 1024
HEAD_DIM = 64
ROPE_BASE = 10000.0
EPS = 1e-6
N_CORES = 8

HPC = N_HEAD // 2        # heads per core = 8
DW = HPC * HEAD_DIM      # local head width = 512
TCH = 512                # token chunk for the fused pipeline
SCALE = 1.0 / np.sqrt(HEAD_DIM)


def build_nc(T=2048, C=2048, num_devices=N_CORES, exchange="cc", reps=1,
             debug_out=False):
    """Build the SPMD program (identical on all cores; data differs).

    exchange: "cc" = pairwise AllGather collectives, "none" = skip exchange
    (timing-only; out-proj reads the local half twice).
    """
    nc = bacc.Bacc("TRN2", target_bir_lowering=False, debug=False,
                   num_devices=num_devices)

    NT = T // 128            # t-tiles = 16
    NCT = C // 128           # contraction tiles for qkv proj = 16
    NJ = T // TCH            # chunks = 4
    TPC = TCH // 128         # t-tiles per chunk = 4
    CH = C // 2              # out c-half width = 1024
    NL = N_LATENT // 128     # latent 128-blocks = 8
    CCW = 512                # out column chunk
    NCC = CH // CCW          # = 2

    x_d = nc.dram_tensor("x", [T, C], F32, kind="ExternalInput").ap()
    wq_d = nc.dram_tensor("wqT", [C, DW], BF16, kind="ExternalInput").ap()
    wk_d = nc.dram_tensor("wkT", [C, DW], BF16, kind="ExternalInput").ap()
    wv_d = nc.dram_tensor("wvT", [C, DW], BF16, kind="ExternalInput").ap()
    wo_d = nc.dram_tensor("woutT", [N_LATENT, CH], BF16,
                          kind="ExternalInput").ap()
    cos_d = nc.dram_tensor("cosf", [T, DW], BF16, kind="ExternalInput").ap()
    sin_d = nc.dram_tensor("sinf", [T, DW], BF16, kind="ExternalInput").ap()
    mask_d = nc.dram_tensor("masks", [4, 128, TCH], BF16,
                            kind="ExternalInput").ap()
    out_d = nc.dram_tensor("out_half", [T, CH], F32, kind="ExternalOutput").ap()
    dbg = {}
    if debug_out:
        for nm, shp in (("qt_o", [DW, T]), ("kt_o", [DW, T]),
                        ("v_o", [T, DW]), ("yt_o", [DW, T])):
            dbg[nm] = nc.dram_tensor(nm, shp, BF16, kind="ExternalOutput").ap()

    groups = [[i, i + 1] for i in range(0, num_devices, 2)]
    exchange_mode = exchange

    with tile.TileContext(nc) as tc:
        with (
            tc.tile_pool(name="const", bufs=1) as constp,
            tc.tile_pool(name="dram", bufs=1, space=bass.MemorySpace.DRAM) as dramp,
            tc.tile_pool(name="wp", bufs=1) as wp,
            tc.tile_pool(name="kvp", bufs=1) as kvp,        # ktt + v65 (persist)
            tc.tile_pool(name="qp", bufs=2) as qp,          # q^T per chunk
            tc.tile_pool(name="p1x", bufs=2) as p1x,        # x staging
            tc.tile_pool(name="p1s", bufs=2) as p1s,        # rope staging
            tc.tile_pool(name="p2s", bufs=2) as p2s,        # pt + att staging
            tc.tile_pool(name="p3y", bufs=1) as p3y,
            tc.tile_pool(name="p3o", bufs=3) as p3o,        # yfull + osb
            tc.tile_pool(name="ps512", bufs=2,
                         space=bass.MemorySpace.PSUM) as ps512,
            tc.tile_pool(name="pssp", bufs=2,
                         space=bass.MemorySpace.PSUM) as pssp,
            tc.tile_pool(name="pysp", bufs=1,
                         space=bass.MemorySpace.PSUM) as pysp,
        ):
            # ---------------- constants + weights ----------------
            eps_sb = constp.tile([128, 1], F32, tag="eps")
            nc.vector.memset(eps_sb[:], EPS)
            mask_sb = constp.tile([128, 4, TCH], BF16, tag="masks")
            nc.sync.dma_start(mask_sb[:], mask_d.rearrange("o p t -> p o t"))

            wsb = {}
            for name, wd in (("q", wq_d), ("k", wk_d), ("v", wv_d)):
                w = wp.tile([128, NCT, DW], BF16, tag=f"w{name}",
                            name=f"w{name}")
                nc.sync.dma_start(
                    w[:], wd.rearrange("(ct p) d -> p ct d", p=128))
                wsb[name] = w
            wo = wp.tile([128, NL, CH], BF16, tag="wo")
            nc.sync.dma_start(wo[:], wo_d.rearrange("(lt p) c -> p lt c",
                                                    p=128))

            # persistent k^T / v65 tiles (one per 128-token block)
            ktts = [kvp.tile([128, HPC // 2, 128], BF16, tag=f"ktt{si}",
                             name=f"ktt{si}") for si in range(NT)]
            v65s = []
            for si in range(NT):
                v = kvp.tile([128, HPC, 65], BF16, tag=f"v65_{si}",
                             name=f"v65_{si}")
                nc.vector.memset(
                    v[:, :, 64:65].rearrange("p h one -> p (h one)"), 1.0)
                v65s.append(v)

            # y^T exchange buffers (DRAM, bf16)
            ytls = [dramp.tile([DW, TCH], BF16, tag=f"ytl{j}",
                               name=f"ytl{j}") for j in range(NJ)]
            ytfs = [dramp.tile([2 * DW, TCH], BF16, tag=f"ytf{j}",
                               name=f"ytf{j}") for j in range(NJ)]
            ytfhs = [dramp.tile([256, TCH], BF16, tag=f"ytfh{hp}",
                                name=f"ytfh{hp}") for hp in range(HPC // 2)]

            qtcs = [None, None]

            # ---------------- phase bodies ----------------
            def p1_tile(tt):
                """QKV + rmsnorm + rope for t-tile tt; fills ktt/v65/qtc."""
                xa = p1x.tile([128, C], F32, tag="xa")
                nc.sync.dma_start(xa[:], x_d[ts(tt, 128), :])
                xab = p1x.tile([128, C], BF16, tag="xab")
                nc.scalar.activation(xab[:], xa[:], AF.Copy)
                xt = p1x.tile([128, NCT, 128], BF16, tag="xt")
                nc.sync.dma_start_transpose(xt[:], xab[:])
                cos_t = p1x.tile([128, DW], BF16, tag="cos")
                sin_t = p1x.tile([128, DW], BF16, tag="sin")
                nc.sync.dma_start(cos_t[:], cos_d[ts(tt, 128), :])
                nc.sync.dma_start(sin_t[:], sin_d[ts(tt, 128), :])

                # q and k processed jointly as [128, 2, DW] (g = q|k)
                qkb = p1s.tile([128, 2, DW], BF16, tag="qkb")
                for gi, name in enumerate(("q", "k", "v")):
                    ps = ps512.tile([128, DW], F32, tag="ps512")
                    for ct in range(NCT):
                        nc.tensor.matmul(
                            ps[:], xt[:, ct, :], wsb[name][:, ct, :],
                            start=(ct == 0), stop=(ct == NCT - 1))
                    if name == "v":
                        nc.scalar.activation(
                            v65s[tt][:, :, 0:64], ps[:], AF.Copy)
                    else:
                        nc.scalar.activation(qkb[:, gi, :], ps[:], AF.Copy)
                # rmsnorm factors: rf = exp(-0.5*ln(ms/64 + eps))
                sq = p1s.tile([128, 2, DW], BF16, tag="sq")
                nc.vector.tensor_mul(sq[:], qkb[:], qkb[:])
                ms = p1s.tile([128, 2 * HPC], F32, tag="ms")
                nc.vector.tensor_reduce(
                    ms[:], sq[:].rearrange("p g (h d) -> p (g h) d",
                                           d=HEAD_DIM),
                    axis=mybir.AxisListType.X, op=mybir.AluOpType.add)
                # rf = rsqrt(ms/64 + eps): Quake seed + 3 Newton steps,
                # all single-op DVE tensor_scalar (validated sequence)
                nc.vector.tensor_scalar(ms[:], ms[:], 1.0 / HEAD_DIM, None,
                                        mybir.AluOpType.mult)
                nc.vector.tensor_scalar(ms[:], ms[:], EPS, None,
                                        mybir.AluOpType.add)
                rfi = p1s.tile([128, 2 * HPC], mybir.dt.int32, tag="rfi")
                nc.vector.tensor_scalar(rfi[:], ms[:].bitcast(mybir.dt.int32),
                                        1, None,
                                        mybir.AluOpType.logical_shift_right)
                nc.vector.tensor_scalar(rfi[:], rfi[:], -1, None,
                                        mybir.AluOpType.mult)
                nc.vector.tensor_scalar(rfi[:], rfi[:], 0x5F3759DF, None,
                                        mybir.AluOpType.add)
                rf = p1s.tile([128, 2 * HPC], F32, tag="rf")
                rfv = rfi[:].bitcast(F32)
                nt = p1s.tile([128, 2 * HPC], F32, tag="nt")
                for _ in range(3):
                    nc.vector.tensor_mul(nt[:], rfv, rfv)
                    nc.vector.tensor_mul(nt[:], nt[:], ms[:])
                    nc.vector.tensor_scalar(nt[:], nt[:], -0.5, None,
                                            mybir.AluOpType.mult)
                    nc.vector.tensor_scalar(nt[:], nt[:], 1.5, None,
                                            mybir.AluOpType.add)
                    nc.vector.tensor_mul(rfv, rfv, nt[:])
                nc.vector.tensor_copy(rf[:], rfv)
                qkn = p1s.tile([128, 2, DW], BF16, tag="qkn")
                rfb = (rf[:].rearrange("p (g h one) -> p g h one", g=2, one=1)
                       .broadcast_to([128, 2, HPC, HEAD_DIM]))
                nc.vector.tensor_mul(
                    qkn[:].rearrange("p g (h d) -> p g h d", d=HEAD_DIM),
                    qkb[:].rearrange("p g (h d) -> p g h d", d=HEAD_DIM), rfb)
                # rope
                qks = p1s.tile([128, 2, DW], BF16, tag="qks")
                hv = qkn[:].rearrange("p g (h two d) -> p g h two d", two=2,
                                      d=HEAD_DIM // 2)
                sv = qks[:].rearrange("p g (h two d) -> p g h two d", two=2,
                                      d=HEAD_DIM // 2)
                nc.vector.tensor_copy(sv[:, :, :, 0, :], hv[:, :, :, 1, :])
                nc.vector.tensor_copy(sv[:, :, :, 1, :], hv[:, :, :, 0, :])
                m1 = p1s.tile([128, 2, DW], BF16, tag="m1")
                m2 = p1s.tile([128, 2, DW], BF16, tag="m2")
                cosb = (cos_t[:].rearrange("p (one d) -> p one d", one=1)
                        .broadcast_to([128, 2, DW]))
                sinb = (sin_t[:].rearrange("p (one d) -> p one d", one=1)
                        .broadcast_to([128, 2, DW]))
                nc.vector.tensor_mul(m1[:], qkn[:], cosb)
                nc.vector.tensor_mul(m2[:], qks[:], sinb)
                nc.vector.tensor_add(m1[:], m1[:], m2[:])
                j, ttl = tt // TPC, tt % TPC
                nc.sync.dma_start_transpose(
                    qtcs[j % 2][:, :, ts(ttl, 128)], m1[:, 0, :])
                nc.sync.dma_start_transpose(ktts[tt][:], m1[:, 1, :])

            def attention(j, hp):
                smax = TPC * (j + 1)
                pys = [pysp.tile([65, TCH], F32, tag=f"py{e}", name=f"py{e}")
                       for e in range(2)]
                for si in range(smax):
                    pss = pssp.tile([128, 2, TCH], F32, tag="pss")
                    for e in range(2):
                        nc.tensor.matmul(
                            pss[:, e, :],
                            ktts[si][ts(e, 64), hp, :],
                            qtcs[j % 2][ts(e, 64), hp, :],
                            start=True, stop=True)
                    pt = p2s.tile([128, 2, TCH], BF16, tag="pt")
                    nc.scalar.activation(
                        pt[:].rearrange("p a b -> p (a b)"),
                        pss[:].rearrange("p a b -> p (a b)"),
                        AF.Exp, scale=SCALE)
                    o = si - (smax - TPC)
                    if o >= 0:
                        for e in range(2):
                            nc.vector.tensor_mul(pt[:, e, :], pt[:, e, :],
                                                 mask_sb[:, o, :])
                    for e in range(2):
                        nc.tensor.matmul(
                            pys[e][:], v65s[si][:, 2 * hp + e, :], pt[:, e, :],
                            start=(si == 0), stop=(si == smax - 1))
                for e in range(2):
                    ystage = p2s.tile([65, TCH], F32, tag="ystage", bufs=2)
                    nc.vector.tensor_copy(ystage[:], pys[e][:])
                    bcr1 = p2s.tile([1, TCH], F32, tag="bcr1", bufs=1)
                    nc.vector.reciprocal(bcr1[:], ystage[64:65, :])
                    bc64 = p2s.tile([64, TCH], F32, tag="bc64", bufs=1)
                    nc.gpsimd.partition_broadcast(bc64[:], bcr1[:])
                    ynt = p2s.tile([64, TCH], BF16, tag="ynt", bufs=2)
                    nc.vector.tensor_mul(ynt[:], ystage[0:64, :], bc64[:])
                    nc.sync.dma_start(
                        ytls[j][ts(2 * hp + e, HEAD_DIM), :], ynt[:])

            def do_exchange(j):
                if exchange_mode == "cc":
                    nc.gpsimd.collective_compute(
                        "AllGather", mybir.AluOpType.bypass,
                        replica_groups=groups,
                        ins=[ytls[j][:]],
                        outs=[ytfs[j][:]])

            def do_exchange_hp(j, hp):
                if exchange_mode == "cc":
                    nc.gpsimd.collective_compute(
                        "AllGather", mybir.AluOpType.bypass,
                        replica_groups=groups,
                        ins=[ytls[j][ts(hp, 128), :]],
                        outs=[ytfhs[hp][:]])

            def p3_chunk(j):
                yf = p3y.tile([128, NL, TCH], BF16, tag="yf")
                if exchange_mode == "cc" and j == NJ - 1:
                    for hp in range(HPC // 2):
                        for h2 in range(2):
                            nc.sync.dma_start(
                                yf[:, h2 * (HPC // 2) + hp, :],
                                ytfhs[hp][ts(h2, 128), :])
                elif exchange_mode == "cc":
                    nc.sync.dma_start(
                        yf[:], ytfs[j][:].rearrange("(lt p) t -> p lt t",
                                                    p=128))
                else:
                    for half in range(2):
                        nc.sync.dma_start(
                            yf[:, half * (NL // 2):(half + 1) * (NL // 2), :],
                            ytls[j][:].rearrange(
                                "(lt p) t -> p lt t", p=128))
                for ttl in range(TPC):
                    tt = j * TPC + ttl
                    for cc in range(NCC):
                        po = ps512.tile([128, CCW], F32, tag="ps512")
                        for lt in range(NL):
                            nc.tensor.matmul(
                                po[:], yf[:, lt, ts(ttl, 128)],
                                wo[:, lt, ts(cc, CCW)],
                                start=(lt == 0), stop=(lt == NL - 1))
                        osb = p3o.tile([128, CCW], F32, tag="osb")
                        nc.vector.tensor_copy(osb[:], po[:])
                        nc.sync.dma_start(out_d[ts(tt, 128), ts(cc, CCW)],
                                          osb[:])

            for _rep in range(reps):
                qtcs[0] = qp.tile([128, HPC // 2, TCH], BF16, tag="qtc0",
                                  name=f"qtc0_{_rep}")
                qtcs[1] = qp.tile([128, HPC // 2, TCH], BF16, tag="qtc1",
                                  name=f"qtc1_{_rep}")
                # Order keeps all DMA-transposes of chunk j+1 ahead of
                # exchange(j): tile serializes collectives against XBAR
                # transposes, so transposes emitted after a collective would
                # stall the P1 pipeline behind it.
                for ttl in range(TPC):
                    p1_tile(ttl)
                for j in range(NJ):
                    for hp in range(HPC // 2):
                        attention(j, hp)
                        if j == NJ - 1:
                            do_exchange_hp(j, hp)
                    if j + 1 < NJ:
                        for ttl in range(TPC):
                            p1_tile((j + 1) * TPC + ttl)
                    if j < NJ - 1:
                        do_exchange(j)
                    # after exchange(j): P3(j-1) depends only on the already-
                    # finished collective(j-1), so it fills the PE hole while
                    # collective(j) blocks chunk j+2's XBAR transposes
                    if j >= 1:
                        p3_chunk(j - 1)
                p3_chunk(NJ - 1)

                if debug_out:
                    for si in range(NT):
                        nc.gpsimd.dma_start(
                            dbg["kt_o"].rearrange(
                                "(hp p) t -> p hp t", p=128)[:, :, ts(si, 128)],
                            ktts[si][:])
                        nc.gpsimd.dma_start(
                            dbg["v_o"][ts(si, 128), :].rearrange(
                                "p (h d) -> p h d", d=HEAD_DIM),
                            v65s[si][:, :, 0:64])
                    for m in range(2):
                        nc.gpsimd.dma_start(
                            dbg["qt_o"].rearrange(
                                "(hp p) t -> p hp t",
                                p=128)[:, :, ts(2 + m, TCH)],
                            qtcs[m][:])
                    for j in range(NJ):
                        nc.gpsimd.dma_start(dbg["yt_o"][:, ts(j, TCH)],
                                            ytls[j][:])

    nc.compile()
    return nc


def host_tables(T=2048):
    inv_freq = 1.0 / (ROPE_BASE ** (np.arange(0, HEAD_DIM, 2, dtype=np.float32)
                                    / HEAD_DIM))
    t = np.arange(T, dtype=np.float32)
    freqs = np.outer(t, inv_freq)
    cos = np.cos(freqs).astype(np.float32)
    sin = np.sin(freqs).astype(np.float32)
    cosf = np.tile(np.concatenate([cos, cos], axis=1), (1, HPC))
    sinf = np.tile(np.concatenate([sin, -sin], axis=1), (1, HPC))
    masks = np.zeros((4, 128, TCH), dtype=np.float32)
    for i, o in enumerate(range(0, TCH, 128)):
        masks[i] = (np.arange(TCH)[None, :] >=
                    (np.arange(128)[:, None] + o)).astype(np.float32)
    bf = ml_dtypes.bfloat16
    return (np.ascontiguousarray(cosf).astype(bf),
            np.ascontiguousarray(sinf).astype(bf),
            masks.astype(bf))


def make_in_maps(x, w_qkv, w_out, T=2048, num_devices=N_CORES):
    bf = ml_dtypes.bfloat16
    x = np.asarray(x, dtype=np.float32)
    w_qkv = np.asarray(w_qkv, dtype=np.float32)
    w_out = np.asarray(w_out, dtype=np.float32)
    C = x.shape[-1]
    cosf, sinf, masks = host_tables(T)
    in_maps = []
    for c in range(num_devices):
        b, hg = c // 2, c % 2
        sl = slice(hg * DW, (hg + 1) * DW)
        in_maps.append({
            "x": np.ascontiguousarray(x[b]),
            "wqT": np.ascontiguousarray(w_qkv[0 * N_LATENT:, :][sl].T).astype(bf),
            "wkT": np.ascontiguousarray(w_qkv[1 * N_LATENT:, :][sl].T).astype(bf),
            "wvT": np.ascontiguousarray(w_qkv[2 * N_LATENT:, :][sl].T).astype(bf),
            "woutT": np.ascontiguousarray(
                w_out[hg * C // 2:(hg + 1) * C // 2, :].T).astype(bf),
            "cosf": cosf,
            "sinf": sinf,
            "masks": masks,
        })
    return in_maps


_NC = None


def kernel(x, w_qkv, w_out):
    global _NC
    if _NC is None:
        _NC = build_nc()
    from concourse.bass_utils import run_bass_kernel_spmd
    in_maps = make_in_maps(x, w_qkv, w_out)
    res = run_bass_kernel_spmd(_NC, in_maps, list(range(N_CORES))).results
    B, T = 4, 2048
    out = np.empty((B, T, N_EMBD), dtype=np.float32)
    for c in range(N_CORES):
        b, hg = c // 2, c % 2
        out[b, :, hg * N_EMBD // 2:(hg + 1) * N_EMBD // 2] = res[c]["out_half"]
    return out
